# revision 28
# baseline (speedup 1.0000x reference)
"""Cached-attention decode step on 8 Trainium2 NeuronCores.

Full-input contract: kernel(**inputs) takes the unsharded inputs (as in
reference.setup_inputs) and returns the full (out, key_t, value_t) tuple.
Internally shards the batch dim (16 batches -> 2 per core) across 8 cores.

Per-core dataflow (memory-bound: ~128MB of cache traffic per core):
  - q/k/v projections on TensorE ([2,1024] @ [1024,1024] per weight).
  - KV cache streamed through SBUF in [128, 1024] tiles (512KB DMAs);
    each tile is written back out (the cache "copy"), with the
    decode_loop_step row updated with k_new/v_new in SBUF.
  - scores = per-head dot(k, q) via VectorE mul + segmented reduce.
  - probs = exp(0.125*scores + mask_adder) on ScalarE (no max-sub needed:
    unmasked scores are O(1); masked get -1e4 -> exp underflows to 0,
    identical ratio to the reference's stable softmax).
  - denominator and (unnormalized) output via TensorE matmuls with the
    probs tile stationary, accumulated in PSUM over the 32 seq tiles.
  - epilogue extracts the per-head diagonal blocks and divides by the sum.
"""

import math

import numpy as np

import concourse.bass as bass
import concourse.mybir as mybir
import concourse.tile as tile
from concourse import bacc
from concourse.bass_utils import run_bass_kernel_spmd

B, F, D, N, H, S = 16, 1, 1024, 16, 64, 4096
NH = N * H            # 1024
NCORES = 8
BPC = B // NCORES     # batches per core
TS = 128              # seq positions per chunk (partition dim)
NT = S // TS          # 32 chunks
CPT = 2               # chunks per DMA tile (1 MiB transfers)
TST = TS * CPT        # 256 seq positions per tile
NT2 = S // TST        # 16 tiles
KC = D // 128         # contraction chunks for projections
NEG_ADDER = -10000.0
SCALE = 1.0 / math.sqrt(float(H))

_BUILD_CACHE: dict[int, object] = {}
LAST_RESULT = None  # BassKernelResults of the most recent run (for profiling)


def _build(step: int):
    f32 = mybir.dt.float32
    i32 = mybir.dt.int32
    ti_step, cc_step, p_step = step // TST, (step % TST) // TS, step % TS

    nc = bacc.Bacc("TRN2", target_bir_lowering=False, debug=False,
                   num_devices=NCORES)

    x_f = nc.dram_tensor("x_f", [BPC, D], f32, kind="ExternalInput")
    x_t = nc.dram_tensor("x_t", [BPC, D], f32, kind="ExternalInput")
    msk = nc.dram_tensor("msk", [BPC, S], i32, kind="ExternalInput")
    ck = nc.dram_tensor("ck", [BPC, S, NH], f32, kind="ExternalInput")
    cv = nc.dram_tensor("cv", [BPC, S, NH], f32, kind="ExternalInput")
    ws = {nm: nc.dram_tensor(nm, [D, NH], f32, kind="ExternalInput")
          for nm in ("wq", "wk", "wv")}
    bs = {nm: nc.dram_tensor(nm, [1, NH], f32, kind="ExternalInput")
          for nm in ("bq", "bk", "bv")}
    q_dram = nc.dram_tensor("q_bounce", [BPC, NH], f32)
    # diag-extract bounce: row n written at flat offset n*(NH+H); its
    # diagonal block (free offset n*H) then sits at n*(NH+2H) + h, so a
    # second view with row stride NH+2H reads the diagonal as columns 0..H.
    # Buffer sized to lcm(NH+H, NH+2H) so both rearrange views divide.
    _WSTR, _RSTR = NH + H, NH + 2 * H       # 1088, 1152
    _OLEN = 19584                           # lcm(1088, 1152) >= 15*1152+64
    o_dram = nc.dram_tensor("o_bounce", [BPC, _OLEN], f32)
    out = nc.dram_tensor("out", [BPC, N, H], f32, kind="ExternalOutput")
    kt = nc.dram_tensor("kt", [BPC, S, NH], f32, kind="ExternalOutput")
    vt = nc.dram_tensor("vt", [BPC, S, NH], f32, kind="ExternalOutput")

    with tile.TileContext(nc) as tc:
        with (
            tc.tile_pool(name="const", bufs=1) as const_pool,
            tc.tile_pool(name="wtile", bufs=6) as w_pool,
            tc.tile_pool(name="projsb", bufs=1) as proj_sb_pool,
            tc.tile_pool(name="projps", bufs=1, space=bass.MemorySpace.PSUM) as proj_psum,
            tc.tile_pool(name="qrep", bufs=2) as qrep_pool,
            tc.tile_pool(name="madd", bufs=2) as madd_pool,
            tc.tile_pool(name="ktile", bufs=6) as k_pool,
            tc.tile_pool(name="vtile", bufs=6) as v_pool,
            tc.tile_pool(name="smul", bufs=3) as smul_pool,
            tc.tile_pool(name="score", bufs=4) as sc_pool,
            tc.tile_pool(name="prob", bufs=4) as pr_pool,
            tc.tile_pool(name="attnps", bufs=2, space=bass.MemorySpace.PSUM) as attn_psum,
            tc.tile_pool(name="denps", bufs=2, space=bass.MemorySpace.PSUM) as den_psum,
            tc.tile_pool(name="outsb", bufs=2) as out_pool,
        ):
            # ---- constants / projections ----
            ones = const_pool.tile([TS, 1], f32)
            nc.vector.memset(ones[:], 1.0)
            xTf = const_pool.tile([128, KC, BPC], f32, tag="xTf")
            xTt = const_pool.tile([128, KC, BPC], f32, tag="xTt")
            for bb in range(BPC):
                nc.sync.dma_start(xTf[:, :, bb],
                                  x_f[bb].rearrange("(c p) -> p c", p=128))
                nc.sync.dma_start(xTt[:, :, bb],
                                  x_t[bb].rearrange("(c p) -> p c", p=128))
            bias_sb = {}
            for nm in ("bq", "bk", "bv"):
                t = const_pool.tile([BPC, NH], f32, tag=nm)
                nc.sync.dma_start(t[:], bs[nm][:].to_broadcast([BPC, NH]))
                bias_sb[nm] = t

            proj = {}
            for wn, bn, xT in (("wq", "bq", xTf), ("wk", "bk", xTt),
                               ("wv", "bv", xTt)):
                sb = proj_sb_pool.tile([BPC, NH], f32, tag=wn)
                ps = proj_psum.tile([BPC, NH], f32)
                for c in range(KC):
                    wt = w_pool.tile([128, NH], f32)
                    nc.sync.dma_start(wt[:], ws[wn][c * 128:(c + 1) * 128, :])
                    for h2 in range(2):
                        nc.tensor.matmul(
                            ps[:, h2 * 512:(h2 + 1) * 512],
                            xT[:, c, :], wt[:, h2 * 512:(h2 + 1) * 512],
                            start=(c == 0), stop=(c == KC - 1))
                nc.vector.tensor_add(sb[:], ps[:], bias_sb[bn][:])
                proj[wn] = sb
            nc.sync.dma_start(q_dram[:], proj["wq"][:])

            # ---- attention + cache copy, batches interleaved ----
            q_reps, adders, ps_os, ps_ds = [], [], [], []
            for b in range(BPC):
                q_rep = qrep_pool.tile([128, NH], f32)
                nc.sync.dma_start(q_rep[:],
                                  q_dram[b:b + 1, :].to_broadcast([128, NH]))
                mi = madd_pool.tile([128, NT], i32, tag="mi")
                nc.sync.dma_start(mi[:], msk[b].rearrange("(c p) -> p c", p=128))
                mf = madd_pool.tile([128, NT], f32, tag="mf")
                nc.vector.tensor_copy(mf[:], mi[:])
                adder = madd_pool.tile([128, NT], f32, tag="adder")
                # (1 - m) * NEG_ADDER == m*(-NEG_ADDER) + NEG_ADDER
                nc.scalar.activation(adder[:], mf[:],
                                     mybir.ActivationFunctionType.Copy,
                                     bias=float(NEG_ADDER), scale=float(-NEG_ADDER))
                q_reps.append(q_rep)
                adders.append(adder)
                ps_os.append(attn_psum.tile([N, NH], f32,
                                            name="ps_o", tag="ps_o"))
                ps_ds.append(den_psum.tile([N, 1], f32,
                                           name="ps_d", tag="ps_d"))

            # step tile last: its copy-out depends on the k/v projections,
            # which would stall the early DMA stream behind the weight loads
            ti_order = [t for t in range(NT2) if t != ti_step] + [ti_step]
            for tn, ti in enumerate(ti_order):
                for b in range(BPC):
                    q_rep, adder = q_reps[b], adders[b]
                    ps_o, ps_d = ps_os[b], ps_ds[b]
                    rows = slice(ti * TST, (ti + 1) * TST)
                    # tile layout [p, cc, nh]: seq t = ti*TST + cc*TS + p
                    k_t = k_pool.tile([128, CPT, NH], f32, tag="k_t")
                    nc.sync.dma_start(
                        k_t[:], ck[b, rows, :].rearrange("(c p) m -> p c m", p=TS))
                    v_t = v_pool.tile([128, CPT, NH], f32, tag="v_t")
                    nc.sync.dma_start(
                        v_t[:], cv[b, rows, :].rearrange("(c p) m -> p c m", p=TS))
                    if ti == ti_step:
                        # step-row update via SWDGE accumulate-DMA (no
                        # partition-alignment constraint, unlike DVE ops)
                        nc.gpsimd.dma_start(k_t[p_step:p_step + 1, cc_step, :],
                                            proj["wk"][b:b + 1, :],
                                            accum_op=mybir.AluOpType.add)
                        nc.gpsimd.dma_start(v_t[p_step:p_step + 1, cc_step, :],
                                            proj["wv"][b:b + 1, :],
                                            accum_op=mybir.AluOpType.add)
                    # cache copy-out (after the step-row update); issued from
                    # the scalar-engine HWDGE ring so a store waiting on its
                    # load can't head-of-line-block the SP ring's loads
                    nc.scalar.dma_start(
                        kt[b, rows, :].rearrange("(c p) m -> p c m", p=TS), k_t[:])
                    nc.scalar.dma_start(
                        vt[b, rows, :].rearrange("(c p) m -> p c m", p=TS), v_t[:])

                    # scores[t, n] = sum_h k[t, n*H+h] * q[n*H+h]
                    sm = smul_pool.tile([128, CPT, NH], f32, tag="sm")
                    nc.vector.tensor_mul(
                        sm[:], k_t[:],
                        q_rep[:, None, :].broadcast_to([128, CPT, NH]))
                    sc = sc_pool.tile([128, CPT, N], f32, tag="sc")
                    nc.vector.reduce_sum(
                        sc[:], sm[:].rearrange("p c (n h) -> p c n h", h=H),
                        axis=mybir.AxisListType.X)
                    pr = pr_pool.tile([128, CPT, N], f32, tag="pr")
                    for cc in range(CPT):
                        nc.scalar.activation(
                            pr[:, cc, :], sc[:, cc, :],
                            mybir.ActivationFunctionType.Exp,
                            bias=adder[:, CPT * ti + cc:CPT * ti + cc + 1],
                            scale=SCALE)
                        # output / denominator accumulation (probs stationary)
                        first = (tn == 0 and cc == 0)
                        last = (tn == NT2 - 1 and cc == CPT - 1)
                        for h2 in range(2):
                            nc.tensor.matmul(
                                ps_o[:, h2 * 512:(h2 + 1) * 512],
                                pr[:, cc, :], v_t[:, cc, h2 * 512:(h2 + 1) * 512],
                                start=first, stop=last)
                        nc.tensor.matmul(ps_d[:], pr[:, cc, :], ones[:],
                                         start=first, stop=last)

            # epilogue: out[n, :] = ps_o[n, n*H:(n+1)*H] / ps_d[n]
            for b in range(BPC):
                ps_o, ps_d = ps_os[b], ps_ds[b]
                rec = out_pool.tile([N, 1], f32, tag="rec")
                nc.vector.reciprocal(rec[:], ps_d[:])
                o_sb = out_pool.tile([N, NH], f32, tag="o_sb")
                nc.scalar.copy(o_sb[:], ps_o[:])
                # normalize the full rows, then gather the diagonal blocks
                # straight into the output with a DRAM->DRAM strided DMA
                nc.vector.tensor_scalar_mul(o_sb[:], o_sb[:], rec[:])
                wview = o_dram[b].rearrange("(a c) -> a c", c=_WSTR)
                nc.sync.dma_start(wview[:N, :NH], o_sb[:])
                rview = o_dram[b].rearrange("(a c) -> a c", c=_RSTR)
                nc.sync.dma_start(out[b], rview[:N, :H])

    nc.compile()
    return nc


def kernel(**inputs):
    global LAST_RESULT
    step = int(np.asarray(inputs["decode_loop_step"]))

    def f32(name, shape):
        return np.ascontiguousarray(
            np.asarray(inputs[name], dtype=np.float32).reshape(shape))

    from_t = f32("from_tensor", (B, D))
    to_t = f32("to_tensor", (B, D))
    mask = np.ascontiguousarray(
        np.asarray(inputs["attention_mask"], dtype=np.int32).reshape(B, S))
    cache_k = f32("cache_key", (B, S, NH))
    cache_v = f32("cache_value", (B, S, NH))
    wq, wk, wv = (f32(nm, (D, NH)) for nm in ("wq", "wk", "wv"))
    bq, bk, bv = (f32(nm, (1, NH)) for nm in ("bq", "bk", "bv"))

    if step not in _BUILD_CACHE:
        _BUILD_CACHE[step] = _build(step)
    nc = _BUILD_CACHE[step]

    in_maps = []
    for c in range(NCORES):
        sl = slice(c * BPC, (c + 1) * BPC)
        in_maps.append({
            "x_f": from_t[sl], "x_t": to_t[sl], "msk": mask[sl],
            "ck": cache_k[sl], "cv": cache_v[sl],
            "wq": wq, "wk": wk, "wv": wv,
            "bq": bq, "bk": bk, "bv": bv,
        })

    res = run_bass_kernel_spmd(nc, in_maps, core_ids=list(range(NCORES)))
    LAST_RESULT = res

    out = np.concatenate([r["out"] for r in res.results], axis=0)
    key_t = np.concatenate([r["kt"] for r in res.results], axis=0)
    value_t = np.concatenate([r["vt"] for r in res.results], axis=0)
    return (out.reshape(B, F, N, H),
            key_t.reshape(B, S, N, H),
            value_t.reshape(B, S, N, H))


# revision 31
# speedup vs baseline: 13.8119x; 13.8119x over previous
"""Cached-attention decode step on 8 Trainium2 NeuronCores.

Batch-sharded attention (2 batches/core) with head-sharded (tensor-
parallel) q/k/v projections:
each core loads only its 2 heads' weight columns (1.5MB instead of 12MB),
computes projections for ALL 16 batches, and an AllToAll hands every core
the full q/k_new/v_new for its own 2 batches. The attention itself stays
batch-sharded. Cuts per-core HBM traffic by ~10.5MB (~7%).
"""

import math

import numpy as np

import concourse.bass as bass
import concourse.mybir as mybir
import concourse.tile as tile
from concourse import bacc
from concourse.bass_utils import run_bass_kernel_spmd

B, F, D, N, H, S = 16, 1, 1024, 16, 64, 4096
NH = N * H            # 1024
NCORES = 8
BPC = B // NCORES     # batches per core
NPC = N // NCORES     # heads per core (projection sharding)
WC = NPC * H          # weight columns per core (128)
TS = 128              # seq positions per chunk (partition dim)
NT = S // TS          # 32 chunks
CPT = 2               # chunks per DMA tile (1 MiB transfers)
TST = TS * CPT        # 256 seq positions per tile
NT2 = S // TST        # 16 tiles
KC = D // 128         # contraction chunks for projections
NEG_ADDER = -10000.0
SCALE = 1.0 / math.sqrt(float(H))

_BUILD_CACHE: dict[int, object] = {}
LAST_RESULT = None  # BassKernelResults of the most recent run (for profiling)


def _build(step: int):
    f32 = mybir.dt.float32
    i32 = mybir.dt.int32
    ti_step, cc_step, p_step = step // TST, (step % TST) // TS, step % TS

    nc = bacc.Bacc("TRN2", target_bir_lowering=False, debug=False,
                   num_devices=NCORES)

    x_f = nc.dram_tensor("x_f", [B, D], f32, kind="ExternalInput")
    x_t = nc.dram_tensor("x_t", [B, D], f32, kind="ExternalInput")
    msk = nc.dram_tensor("msk", [BPC, S], i32, kind="ExternalInput")
    ck = nc.dram_tensor("ck", [BPC, S, NH], f32, kind="ExternalInput")
    cv = nc.dram_tensor("cv", [BPC, S, NH], f32, kind="ExternalInput")
    ws = {nm: nc.dram_tensor(nm, [D, WC], f32, kind="ExternalInput")
          for nm in ("wq", "wk", "wv")}
    bs = {nm: nc.dram_tensor(nm, [1, WC], f32, kind="ExternalInput")
          for nm in ("bq", "bk", "bv")}
    # diag-extract bounce: row n written at flat offset n*(NH+H); its
    # diagonal block (free offset n*H) then sits at n*(NH+2H) + h, so a
    # second view with row stride NH+2H reads the diagonal as columns 0..H.
    # Buffer sized to lcm(NH+H, NH+2H) so both rearrange views divide.
    _WSTR, _RSTR = NH + H, NH + 2 * H       # 1088, 1152
    _OLEN = 19584                           # lcm(1088, 1152) >= 15*1152+64
    o_dram = nc.dram_tensor("o_bounce", [BPC, _OLEN], f32)
    out = nc.dram_tensor("out", [BPC, N, H], f32, kind="ExternalOutput")
    kt = nc.dram_tensor("kt", [BPC, S, NH], f32, kind="ExternalOutput")
    vt = nc.dram_tensor("vt", [BPC, S, NH], f32, kind="ExternalOutput")

    with tile.TileContext(nc) as tc:
        with (
            tc.tile_pool(name="const", bufs=1) as const_pool,
            tc.tile_pool(name="wtile", bufs=3) as w_pool,
            tc.tile_pool(name="projsb", bufs=1) as proj_sb_pool,
            tc.tile_pool(name="projps", bufs=2, space=bass.MemorySpace.PSUM) as proj_psum,
            tc.tile_pool(name="dram", bufs=1, space="DRAM") as dram_pool,
            tc.tile_pool(name="qrep", bufs=2) as qrep_pool,
            tc.tile_pool(name="madd", bufs=2) as madd_pool,
            tc.tile_pool(name="ktile", bufs=6) as k_pool,
            tc.tile_pool(name="vtile", bufs=6) as v_pool,
            tc.tile_pool(name="smul", bufs=3) as smul_pool,
            tc.tile_pool(name="score", bufs=4) as sc_pool,
            tc.tile_pool(name="prob", bufs=4) as pr_pool,
            tc.tile_pool(name="attnps", bufs=2, space=bass.MemorySpace.PSUM) as attn_psum,
            tc.tile_pool(name="denps", bufs=2, space=bass.MemorySpace.PSUM) as den_psum,
            tc.tile_pool(name="outsb", bufs=2) as out_pool,
        ):
            # ---- constants ----
            ones = const_pool.tile([TS, 1], f32)
            nc.vector.memset(ones[:], 1.0)
            xTf = const_pool.tile([128, KC, B], f32, tag="xTf")
            xTt = const_pool.tile([128, KC, B], f32, tag="xTt")
            for bb in range(B):
                nc.sync.dma_start(xTf[:, :, bb],
                                  x_f[bb].rearrange("(c p) -> p c", p=128))
                nc.sync.dma_start(xTt[:, :, bb],
                                  x_t[bb].rearrange("(c p) -> p c", p=128))
            bias_sb = {}
            for nm in ("bq", "bk", "bv"):
                t = const_pool.tile([B, WC], f32, tag=nm)
                nc.sync.dma_start(t[:], bs[nm][:].to_broadcast([B, WC]))
                bias_sb[nm] = t

            # ---- head-sharded projections for ALL batches ----
            # ccin rows are global batches = (dest_core, local_batch) — already
            # destination-major, so a flat AllToAll hands every core exactly
            # its own batches' projections from all 8 head-blocks.
            ccin_d = dram_pool.tile([B, 3 * WC], f32, name="ccin_d")
            gout_d = dram_pool.tile([B, 3 * WC], f32, name="gout_d")
            for wi, (wn, bn, xT) in enumerate(
                    (("wq", "bq", xTf), ("wk", "bk", xTt), ("wv", "bv", xTt))):
                wt = w_pool.tile([128, KC, WC], f32, tag="wt")
                nc.sync.dma_start(
                    wt[:], ws[wn][:].rearrange("(c p) m -> p c m", p=128))
                ps = proj_psum.tile([B, WC], f32, name="ps", tag="ps")
                for c in range(KC):
                    nc.tensor.matmul(ps[:], xT[:, c, :], wt[:, c, :],
                                     start=(c == 0), stop=(c == KC - 1))
                sb = proj_sb_pool.tile([B, WC], f32, name="sb", tag=wn)
                nc.vector.tensor_add(sb[:], ps[:], bias_sb[bn][:])
                nc.sync.dma_start(ccin_d[:, wi * WC:(wi + 1) * WC], sb[:])

            nc.gpsimd.collective_compute(
                "AllToAll", mybir.AluOpType.bypass,
                replica_groups=[list(range(NCORES))],
                ins=[ccin_d[:].opt()], outs=[gout_d[:].opt()])
            # received rows are (src_head_block, local_batch);
            # view as [local_batch, head_block, 3*WC]
            gv = gout_d[:].rearrange("(r b) m -> b r m", b=BPC)

            # ---- attention + cache copy, batches interleaved ----
            q_reps, adders, ps_os, ps_ds = [], [], [], []
            for b in range(BPC):
                q_rep = qrep_pool.tile([128, NH], f32)
                nc.sync.dma_start(
                    q_rep[:].rearrange("p (r c) -> p r c", c=WC),
                    gv[b:b + 1, :, 0:WC].to_broadcast([128, NCORES, WC]))
                mi = madd_pool.tile([128, NT], i32, tag="mi")
                nc.sync.dma_start(mi[:], msk[b].rearrange("(c p) -> p c", p=128))
                mf = madd_pool.tile([128, NT], f32, tag="mf")
                nc.vector.tensor_copy(mf[:], mi[:])
                adder = madd_pool.tile([128, NT], f32, tag="adder")
                # (1 - m) * NEG_ADDER == m*(-NEG_ADDER) + NEG_ADDER
                nc.scalar.activation(adder[:], mf[:],
                                     mybir.ActivationFunctionType.Copy,
                                     bias=float(NEG_ADDER), scale=float(-NEG_ADDER))
                q_reps.append(q_rep)
                adders.append(adder)
                ps_os.append(attn_psum.tile([N, NH], f32,
                                            name="ps_o", tag="ps_o"))
                ps_ds.append(den_psum.tile([N, 1], f32,
                                           name="ps_d", tag="ps_d"))

            def epilogue(b):
                # out[n, :] = ps_o[n, n*H:(n+1)*H] / ps_d[n]
                ps_o, ps_d = ps_os[b], ps_ds[b]
                rec = out_pool.tile([N, 1], f32, name="rec", tag="rec")
                nc.vector.reciprocal(rec[:], ps_d[:])
                o_sb = out_pool.tile([N, NH], f32, name="o_sb", tag="o_sb")
                nc.scalar.copy(o_sb[:], ps_o[:])
                # normalize the full rows, then gather the diagonal blocks
                # straight into the output with a DRAM->DRAM strided DMA
                nc.vector.tensor_scalar_mul(o_sb[:], o_sb[:], rec[:])
                wview = o_dram[b].rearrange("(a c) -> a c", c=_WSTR)
                nc.sync.dma_start(wview[:N, :NH], o_sb[:])
                rview = o_dram[b].rearrange("(a c) -> a c", c=_RSTR)
                nc.sync.dma_start(out[b], rview[:N, :H])

            # step tile first: its copy-out depends on the AllToAll (as does
            # every tile's score path), and putting it early keeps that
            # latency off the kernel tail where it would serialize.
            # Batch 1 runs one slot behind batch 0 so batch 0's epilogue
            # overlaps batch 1's last tile.
            ti_order = [ti_step] + [t for t in range(NT2) if t != ti_step]
            sched = []
            for i in range(NT2 + 1):
                if i < NT2:
                    sched.append((i, 0))
                if i >= 1:
                    sched.append((i - 1, 1))
            for tn, b in sched:
                    ti = ti_order[tn]
                    q_rep, adder = q_reps[b], adders[b]
                    ps_o, ps_d = ps_os[b], ps_ds[b]
                    rows = slice(ti * TST, (ti + 1) * TST)
                    # tile layout [p, cc, nh]: seq t = ti*TST + cc*TS + p
                    k_t = k_pool.tile([128, CPT, NH], f32, tag="k_t")
                    nc.sync.dma_start(
                        k_t[:], ck[b, rows, :].rearrange("(c p) m -> p c m", p=TS))
                    v_t = v_pool.tile([128, CPT, NH], f32, tag="v_t")
                    nc.sync.dma_start(
                        v_t[:], cv[b, rows, :].rearrange("(c p) m -> p c m", p=TS))
                    if ti == ti_step:
                        # step-row update via SWDGE accumulate-DMA (no
                        # partition-alignment constraint, unlike DVE ops)
                        nc.gpsimd.dma_start(
                            k_t[p_step:p_step + 1, cc_step, :],
                            gv[b:b + 1, :, WC:2 * WC],
                            accum_op=mybir.AluOpType.add)
                        nc.gpsimd.dma_start(
                            v_t[p_step:p_step + 1, cc_step, :],
                            gv[b:b + 1, :, 2 * WC:3 * WC],
                            accum_op=mybir.AluOpType.add)
                    # cache copy-out (after the step-row update); issued from
                    # the scalar-engine HWDGE ring so a store waiting on its
                    # load can't head-of-line-block the SP ring's loads
                    nc.scalar.dma_start(
                        kt[b, rows, :].rearrange("(c p) m -> p c m", p=TS), k_t[:])
                    nc.scalar.dma_start(
                        vt[b, rows, :].rearrange("(c p) m -> p c m", p=TS), v_t[:])

                    # scores[t, n] = sum_h k[t, n*H+h] * q[n*H+h]
                    sm = smul_pool.tile([128, CPT, NH], f32, tag="sm")
                    nc.vector.tensor_mul(
                        sm[:], k_t[:],
                        q_rep[:, None, :].broadcast_to([128, CPT, NH]))
                    sc = sc_pool.tile([128, CPT, N], f32, tag="sc")
                    nc.vector.reduce_sum(
                        sc[:], sm[:].rearrange("p c (n h) -> p c n h", h=H),
                        axis=mybir.AxisListType.X)
                    pr = pr_pool.tile([128, CPT, N], f32, tag="pr")
                    for cc in range(CPT):
                        nc.scalar.activation(
                            pr[:, cc, :], sc[:, cc, :],
                            mybir.ActivationFunctionType.Exp,
                            bias=adder[:, CPT * ti + cc:CPT * ti + cc + 1],
                            scale=SCALE)
                        # output / denominator accumulation (probs stationary)
                        first = (tn == 0 and cc == 0)
                        last = (tn == NT2 - 1 and cc == CPT - 1)
                        for h2 in range(2):
                            nc.tensor.matmul(
                                ps_o[:, h2 * 512:(h2 + 1) * 512],
                                pr[:, cc, :], v_t[:, cc, h2 * 512:(h2 + 1) * 512],
                                start=first, stop=last)
                        nc.tensor.matmul(ps_d[:], pr[:, cc, :], ones[:],
                                         start=first, stop=last)
                    if tn == NT2 - 1:
                        epilogue(b)

    nc.compile()
    return nc


def kernel(**inputs):
    global LAST_RESULT
    step = int(np.asarray(inputs["decode_loop_step"]))

    def f32(name, shape):
        return np.ascontiguousarray(
            np.asarray(inputs[name], dtype=np.float32).reshape(shape))

    from_t = f32("from_tensor", (B, D))
    to_t = f32("to_tensor", (B, D))
    mask = np.ascontiguousarray(
        np.asarray(inputs["attention_mask"], dtype=np.int32).reshape(B, S))
    cache_k = f32("cache_key", (B, S, NH))
    cache_v = f32("cache_value", (B, S, NH))
    wq, wk, wv = (f32(nm, (D, NH)) for nm in ("wq", "wk", "wv"))
    bq, bk, bv = (f32(nm, (1, NH)) for nm in ("bq", "bk", "bv"))

    if step not in _BUILD_CACHE:
        _BUILD_CACHE[step] = _build(step)
    nc = _BUILD_CACHE[step]

    in_maps = []
    for c in range(NCORES):
        sl = slice(c * BPC, (c + 1) * BPC)
        wsl = slice(c * WC, (c + 1) * WC)
        in_maps.append({
            "x_f": from_t, "x_t": to_t, "msk": mask[sl],
            "ck": cache_k[sl], "cv": cache_v[sl],
            "wq": np.ascontiguousarray(wq[:, wsl]),
            "wk": np.ascontiguousarray(wk[:, wsl]),
            "wv": np.ascontiguousarray(wv[:, wsl]),
            "bq": np.ascontiguousarray(bq[:, wsl]),
            "bk": np.ascontiguousarray(bk[:, wsl]),
            "bv": np.ascontiguousarray(bv[:, wsl]),
        })

    res = run_bass_kernel_spmd(nc, in_maps, core_ids=list(range(NCORES)))
    LAST_RESULT = res

    out = np.concatenate([r["out"] for r in res.results], axis=0)
    key_t = np.concatenate([r["kt"] for r in res.results], axis=0)
    value_t = np.concatenate([r["vt"] for r in res.results], axis=0)
    return (out.reshape(B, F, N, H),
            key_t.reshape(B, S, N, H),
            value_t.reshape(B, S, N, H))


# revision 32
# speedup vs baseline: 13.9070x; 1.0069x over previous
"""Cached-attention decode step on 8 Trainium2 NeuronCores.

Batch-sharded attention (2 batches/core) with head-sharded (tensor-
parallel) q/k/v projections:
each core loads only its 2 heads' weight columns (1.5MB instead of 12MB),
computes projections for ALL 16 batches, and an AllToAll hands every core
the full q/k_new/v_new for its own 2 batches. The attention itself stays
batch-sharded. Cuts per-core HBM traffic by ~10.5MB (~7%).
"""

import math

import numpy as np

import concourse.bass as bass
import concourse.mybir as mybir
import concourse.tile as tile
from concourse import bacc
from concourse.bass_utils import run_bass_kernel_spmd

B, F, D, N, H, S = 16, 1, 1024, 16, 64, 4096
NH = N * H            # 1024
NCORES = 8
BPC = B // NCORES     # batches per core
NPC = N // NCORES     # heads per core (projection sharding)
WC = NPC * H          # weight columns per core (128)
TS = 128              # seq positions per chunk (partition dim)
NT = S // TS          # 32 chunks
CPT = 2               # chunks per DMA tile (1 MiB transfers)
TST = TS * CPT        # 256 seq positions per tile
NT2 = S // TST        # 16 tiles
KC = D // 128         # contraction chunks for projections
NEG_ADDER = -10000.0
SCALE = 1.0 / math.sqrt(float(H))

_BUILD_CACHE: dict[int, object] = {}
LAST_RESULT = None  # BassKernelResults of the most recent run (for profiling)


def _build(step: int):
    f32 = mybir.dt.float32
    i32 = mybir.dt.int32
    ti_step, cc_step, p_step = step // TST, (step % TST) // TS, step % TS

    nc = bacc.Bacc("TRN2", target_bir_lowering=False, debug=False,
                   num_devices=NCORES)

    x_f = nc.dram_tensor("x_f", [B, D], f32, kind="ExternalInput")
    x_t = nc.dram_tensor("x_t", [B, D], f32, kind="ExternalInput")
    msk = nc.dram_tensor("msk", [BPC, S], i32, kind="ExternalInput")
    ck = nc.dram_tensor("ck", [BPC, S, NH], f32, kind="ExternalInput")
    cv = nc.dram_tensor("cv", [BPC, S, NH], f32, kind="ExternalInput")
    ws = {nm: nc.dram_tensor(nm, [D, WC], f32, kind="ExternalInput")
          for nm in ("wq", "wk", "wv")}
    bs = {nm: nc.dram_tensor(nm, [1, WC], f32, kind="ExternalInput")
          for nm in ("bq", "bk", "bv")}
    # diag-extract bounce: row n written at flat offset n*(NH+H); its
    # diagonal block (free offset n*H) then sits at n*(NH+2H) + h, so a
    # second view with row stride NH+2H reads the diagonal as columns 0..H.
    # Buffer sized to lcm(NH+H, NH+2H) so both rearrange views divide.
    _WSTR, _RSTR = NH + H, NH + 2 * H       # 1088, 1152
    _OLEN = 19584                           # lcm(1088, 1152) >= 15*1152+64
    o_dram = nc.dram_tensor("o_bounce", [BPC, _OLEN], f32)
    out = nc.dram_tensor("out", [BPC, N, H], f32, kind="ExternalOutput")
    kt = nc.dram_tensor("kt", [BPC, S, NH], f32, kind="ExternalOutput")
    vt = nc.dram_tensor("vt", [BPC, S, NH], f32, kind="ExternalOutput")

    with tile.TileContext(nc) as tc:
        with (
            tc.tile_pool(name="const", bufs=1) as const_pool,
            tc.tile_pool(name="wtile", bufs=3) as w_pool,
            tc.tile_pool(name="projsb", bufs=1) as proj_sb_pool,
            tc.tile_pool(name="projps", bufs=2, space=bass.MemorySpace.PSUM) as proj_psum,
            tc.tile_pool(name="dram", bufs=1, space="DRAM") as dram_pool,
            tc.tile_pool(name="qrep", bufs=2) as qrep_pool,
            tc.tile_pool(name="madd", bufs=2) as madd_pool,
            tc.tile_pool(name="ktile", bufs=6) as k_pool,
            tc.tile_pool(name="vtile", bufs=6) as v_pool,
            tc.tile_pool(name="smul", bufs=3) as smul_pool,
            tc.tile_pool(name="score", bufs=4) as sc_pool,
            tc.tile_pool(name="prob", bufs=4) as pr_pool,
            tc.tile_pool(name="attnps", bufs=2, space=bass.MemorySpace.PSUM) as attn_psum,
            tc.tile_pool(name="denps", bufs=2, space=bass.MemorySpace.PSUM) as den_psum,
            tc.tile_pool(name="outsb", bufs=2) as out_pool,
        ):
            # ---- constants ----
            ones = const_pool.tile([TS, 1], f32)
            nc.vector.memset(ones[:], 1.0)
            xTf = const_pool.tile([128, KC, B], f32, tag="xTf")
            xTt = const_pool.tile([128, KC, B], f32, tag="xTt")
            for bb in range(B):
                nc.sync.dma_start(xTf[:, :, bb],
                                  x_f[bb].rearrange("(c p) -> p c", p=128))
                nc.sync.dma_start(xTt[:, :, bb],
                                  x_t[bb].rearrange("(c p) -> p c", p=128))
            bias_sb = {}
            for nm in ("bq", "bk", "bv"):
                t = const_pool.tile([B, WC], f32, tag=nm)
                nc.sync.dma_start(t[:], bs[nm][:].to_broadcast([B, WC]))
                bias_sb[nm] = t

            # ---- head-sharded projections for ALL batches ----
            # ccin rows are global batches = (dest_core, local_batch) — already
            # destination-major, so a flat AllToAll hands every core exactly
            # its own batches' projections from all 8 head-blocks.
            ccin_d = dram_pool.tile([B, 3 * WC], f32, name="ccin_d")
            gout_d = dram_pool.tile([B, 3 * WC], f32, name="gout_d")
            for wi, (wn, bn, xT) in enumerate(
                    (("wq", "bq", xTf), ("wk", "bk", xTt), ("wv", "bv", xTt))):
                wt = w_pool.tile([128, KC, WC], f32, tag="wt")
                nc.sync.dma_start(
                    wt[:], ws[wn][:].rearrange("(c p) m -> p c m", p=128))
                ps = proj_psum.tile([B, WC], f32, name="ps", tag="ps")
                for c in range(KC):
                    nc.tensor.matmul(ps[:], xT[:, c, :], wt[:, c, :],
                                     start=(c == 0), stop=(c == KC - 1))
                sb = proj_sb_pool.tile([B, WC], f32, name="sb", tag=wn)
                nc.vector.tensor_add(sb[:], ps[:], bias_sb[bn][:])
                nc.sync.dma_start(ccin_d[:, wi * WC:(wi + 1) * WC], sb[:])

            nc.gpsimd.collective_compute(
                "AllToAll", mybir.AluOpType.bypass,
                replica_groups=[list(range(NCORES))],
                ins=[ccin_d[:].opt()], outs=[gout_d[:].opt()])
            # received rows are (src_head_block, local_batch);
            # view as [local_batch, head_block, 3*WC]
            gv = gout_d[:].rearrange("(r b) m -> b r m", b=BPC)

            # ---- attention + cache copy, batches interleaved ----
            q_reps, adders, ps_os, ps_ds = [], [], [], []
            for b in range(BPC):
                q_rep = qrep_pool.tile([128, NH], f32)
                nc.sync.dma_start(
                    q_rep[:].rearrange("p (r c) -> p r c", c=WC),
                    gv[b:b + 1, :, 0:WC].to_broadcast([128, NCORES, WC]))
                mi = madd_pool.tile([128, NT], i32, tag="mi")
                nc.sync.dma_start(mi[:], msk[b].rearrange("(c p) -> p c", p=128))
                mf = madd_pool.tile([128, NT], f32, tag="mf")
                nc.vector.tensor_copy(mf[:], mi[:])
                adder = madd_pool.tile([128, NT], f32, tag="adder")
                # (1 - m) * NEG_ADDER == m*(-NEG_ADDER) + NEG_ADDER
                nc.scalar.activation(adder[:], mf[:],
                                     mybir.ActivationFunctionType.Copy,
                                     bias=float(NEG_ADDER), scale=float(-NEG_ADDER))
                q_reps.append(q_rep)
                adders.append(adder)
                ps_os.append(attn_psum.tile([N, NH], f32,
                                            name="ps_o", tag="ps_o"))
                ps_ds.append(den_psum.tile([N, 1], f32,
                                           name="ps_d", tag="ps_d"))

            def epilogue(b):
                # out[n, :] = ps_o[n, n*H:(n+1)*H] / ps_d[n]
                ps_o, ps_d = ps_os[b], ps_ds[b]
                rec = out_pool.tile([N, 1], f32, name="rec", tag="rec")
                nc.vector.reciprocal(rec[:], ps_d[:])
                # copy out of PSUM fused with the 1/denom normalization
                # (ACT Copy with per-partition scale), then gather the
                # diagonal blocks into the output with a DRAM->DRAM
                # strided DMA
                o_sb = out_pool.tile([N, NH], f32, name="o_sb", tag="o_sb")
                nc.scalar.mul(o_sb[:], ps_o[:], rec[:])
                wview = o_dram[b].rearrange("(a c) -> a c", c=_WSTR)
                nc.sync.dma_start(wview[:N, :NH], o_sb[:])
                rview = o_dram[b].rearrange("(a c) -> a c", c=_RSTR)
                nc.sync.dma_start(out[b], rview[:N, :H])

            # step tile first: its copy-out depends on the AllToAll (as does
            # every tile's score path), and putting it early keeps that
            # latency off the kernel tail where it would serialize.
            # Batch 1 runs one slot behind batch 0 so batch 0's epilogue
            # overlaps batch 1's last tile.
            ti_order = [ti_step] + [t for t in range(NT2) if t != ti_step]
            sched = []
            for i in range(NT2 + 1):
                if i < NT2:
                    sched.append((i, 0))
                if i >= 1:
                    sched.append((i - 1, 1))
            for tn, b in sched:
                    ti = ti_order[tn]
                    q_rep, adder = q_reps[b], adders[b]
                    ps_o, ps_d = ps_os[b], ps_ds[b]
                    rows = slice(ti * TST, (ti + 1) * TST)
                    # tile layout [p, cc, nh]: seq t = ti*TST + cc*TS + p
                    k_t = k_pool.tile([128, CPT, NH], f32, tag="k_t")
                    nc.sync.dma_start(
                        k_t[:], ck[b, rows, :].rearrange("(c p) m -> p c m", p=TS))
                    v_t = v_pool.tile([128, CPT, NH], f32, tag="v_t")
                    nc.sync.dma_start(
                        v_t[:], cv[b, rows, :].rearrange("(c p) m -> p c m", p=TS))
                    if ti == ti_step:
                        # step-row update via SWDGE accumulate-DMA (no
                        # partition-alignment constraint, unlike DVE ops)
                        nc.gpsimd.dma_start(
                            k_t[p_step:p_step + 1, cc_step, :],
                            gv[b:b + 1, :, WC:2 * WC],
                            accum_op=mybir.AluOpType.add)
                        nc.gpsimd.dma_start(
                            v_t[p_step:p_step + 1, cc_step, :],
                            gv[b:b + 1, :, 2 * WC:3 * WC],
                            accum_op=mybir.AluOpType.add)
                    # cache copy-out (after the step-row update); issued from
                    # the scalar-engine HWDGE ring so a store waiting on its
                    # load can't head-of-line-block the SP ring's loads
                    nc.scalar.dma_start(
                        kt[b, rows, :].rearrange("(c p) m -> p c m", p=TS), k_t[:])
                    nc.scalar.dma_start(
                        vt[b, rows, :].rearrange("(c p) m -> p c m", p=TS), v_t[:])

                    # scores[t, n] = sum_h k[t, n*H+h] * q[n*H+h]
                    sm = smul_pool.tile([128, CPT, NH], f32, tag="sm")
                    nc.vector.tensor_mul(
                        sm[:], k_t[:],
                        q_rep[:, None, :].broadcast_to([128, CPT, NH]))
                    sc = sc_pool.tile([128, CPT, N], f32, tag="sc")
                    nc.vector.reduce_sum(
                        sc[:], sm[:].rearrange("p c (n h) -> p c n h", h=H),
                        axis=mybir.AxisListType.X)
                    pr = pr_pool.tile([128, CPT, N], f32, tag="pr")
                    for cc in range(CPT):
                        nc.scalar.activation(
                            pr[:, cc, :], sc[:, cc, :],
                            mybir.ActivationFunctionType.Exp,
                            bias=adder[:, CPT * ti + cc:CPT * ti + cc + 1],
                            scale=SCALE)
                        # output / denominator accumulation (probs stationary)
                        first = (tn == 0 and cc == 0)
                        last = (tn == NT2 - 1 and cc == CPT - 1)
                        for h2 in range(2):
                            nc.tensor.matmul(
                                ps_o[:, h2 * 512:(h2 + 1) * 512],
                                pr[:, cc, :], v_t[:, cc, h2 * 512:(h2 + 1) * 512],
                                start=first, stop=last)
                        nc.tensor.matmul(ps_d[:], pr[:, cc, :], ones[:],
                                         start=first, stop=last)
                    if tn == NT2 - 1:
                        epilogue(b)

    nc.compile()
    return nc


def kernel(**inputs):
    global LAST_RESULT
    step = int(np.asarray(inputs["decode_loop_step"]))

    def f32(name, shape):
        return np.ascontiguousarray(
            np.asarray(inputs[name], dtype=np.float32).reshape(shape))

    from_t = f32("from_tensor", (B, D))
    to_t = f32("to_tensor", (B, D))
    mask = np.ascontiguousarray(
        np.asarray(inputs["attention_mask"], dtype=np.int32).reshape(B, S))
    cache_k = f32("cache_key", (B, S, NH))
    cache_v = f32("cache_value", (B, S, NH))
    wq, wk, wv = (f32(nm, (D, NH)) for nm in ("wq", "wk", "wv"))
    bq, bk, bv = (f32(nm, (1, NH)) for nm in ("bq", "bk", "bv"))

    if step not in _BUILD_CACHE:
        _BUILD_CACHE[step] = _build(step)
    nc = _BUILD_CACHE[step]

    in_maps = []
    for c in range(NCORES):
        sl = slice(c * BPC, (c + 1) * BPC)
        wsl = slice(c * WC, (c + 1) * WC)
        in_maps.append({
            "x_f": from_t, "x_t": to_t, "msk": mask[sl],
            "ck": cache_k[sl], "cv": cache_v[sl],
            "wq": np.ascontiguousarray(wq[:, wsl]),
            "wk": np.ascontiguousarray(wk[:, wsl]),
            "wv": np.ascontiguousarray(wv[:, wsl]),
            "bq": np.ascontiguousarray(bq[:, wsl]),
            "bk": np.ascontiguousarray(bk[:, wsl]),
            "bv": np.ascontiguousarray(bv[:, wsl]),
        })

    res = run_bass_kernel_spmd(nc, in_maps, core_ids=list(range(NCORES)))
    LAST_RESULT = res

    out = np.concatenate([r["out"] for r in res.results], axis=0)
    key_t = np.concatenate([r["kt"] for r in res.results], axis=0)
    value_t = np.concatenate([r["vt"] for r in res.results], axis=0)
    return (out.reshape(B, F, N, H),
            key_t.reshape(B, S, N, H),
            value_t.reshape(B, S, N, H))


# revision 33
# speedup vs baseline: 14.1648x; 1.0185x over previous
"""Cached-attention decode step on 8 Trainium2 NeuronCores.

Batch-sharded attention (2 batches/core) with head-sharded (tensor-
parallel) q/k/v projections:
each core loads only its 2 heads' weight columns (1.5MB instead of 12MB),
computes projections for ALL 16 batches, and an AllToAll hands every core
the full q/k_new/v_new for its own 2 batches. The attention itself stays
batch-sharded. Cuts per-core HBM traffic by ~10.5MB (~7%).
"""

import math

import numpy as np

import concourse.bass as bass
import concourse.mybir as mybir
import concourse.tile as tile
from concourse import bacc
from concourse.bass_utils import run_bass_kernel_spmd

B, F, D, N, H, S = 16, 1, 1024, 16, 64, 4096
NH = N * H            # 1024
NCORES = 8
BPC = B // NCORES     # batches per core
NPC = N // NCORES     # heads per core (projection sharding)
WC = NPC * H          # weight columns per core (128)
TS = 128              # seq positions per chunk (partition dim)
NT = S // TS          # 32 chunks
CPT = 2               # chunks per DMA tile (1 MiB transfers)
TST = TS * CPT        # 256 seq positions per tile
NT2 = S // TST        # 16 tiles
KC = D // 128         # contraction chunks for projections
NEG_ADDER = -10000.0
SCALE = 1.0 / math.sqrt(float(H))

_BUILD_CACHE: dict[int, object] = {}
LAST_RESULT = None  # BassKernelResults of the most recent run (for profiling)


def _build(step: int):
    f32 = mybir.dt.float32
    i32 = mybir.dt.int32
    ti_step, cc_step, p_step = step // TST, (step % TST) // TS, step % TS

    nc = bacc.Bacc("TRN2", target_bir_lowering=False, debug=False,
                   num_devices=NCORES)

    x_f = nc.dram_tensor("x_f", [B, D], f32, kind="ExternalInput")
    x_t = nc.dram_tensor("x_t", [B, D], f32, kind="ExternalInput")
    msk = nc.dram_tensor("msk", [BPC, S], i32, kind="ExternalInput")
    ck = nc.dram_tensor("ck", [BPC, S, NH], f32, kind="ExternalInput")
    cv = nc.dram_tensor("cv", [BPC, S, NH], f32, kind="ExternalInput")
    ws = {nm: nc.dram_tensor(nm, [D, WC], f32, kind="ExternalInput")
          for nm in ("wq", "wk", "wv")}
    bs = {nm: nc.dram_tensor(nm, [1, WC], f32, kind="ExternalInput")
          for nm in ("bq", "bk", "bv")}
    # diag-extract bounce: row n written at flat offset n*(NH+H); its
    # diagonal block (free offset n*H) then sits at n*(NH+2H) + h, so a
    # second view with row stride NH+2H reads the diagonal as columns 0..H.
    # Buffer sized to lcm(NH+H, NH+2H) so both rearrange views divide.
    _WSTR, _RSTR = NH + H, NH + 2 * H       # 1088, 1152
    _OLEN = 19584                           # lcm(1088, 1152) >= 15*1152+64
    o_dram = nc.dram_tensor("o_bounce", [BPC, _OLEN], f32)
    out = nc.dram_tensor("out", [BPC, N, H], f32, kind="ExternalOutput")
    kt = nc.dram_tensor("kt", [BPC, S, NH], f32, kind="ExternalOutput")
    vt = nc.dram_tensor("vt", [BPC, S, NH], f32, kind="ExternalOutput")

    with tile.TileContext(nc) as tc:
        with (
            tc.tile_pool(name="const", bufs=1) as const_pool,
            tc.tile_pool(name="wtile", bufs=3) as w_pool,
            tc.tile_pool(name="projsb", bufs=1) as proj_sb_pool,
            tc.tile_pool(name="projps", bufs=2, space=bass.MemorySpace.PSUM) as proj_psum,
            tc.tile_pool(name="dram", bufs=1, space="DRAM") as dram_pool,
            tc.tile_pool(name="qrep", bufs=2) as qrep_pool,
            tc.tile_pool(name="madd", bufs=2) as madd_pool,
            tc.tile_pool(name="ktile", bufs=6) as k_pool,
            tc.tile_pool(name="vtile", bufs=6) as v_pool,
            tc.tile_pool(name="smul", bufs=3) as smul_pool,
            tc.tile_pool(name="score", bufs=4) as sc_pool,
            tc.tile_pool(name="prob", bufs=4) as pr_pool,
            tc.tile_pool(name="attnps", bufs=2, space=bass.MemorySpace.PSUM) as attn_psum,
            tc.tile_pool(name="denps", bufs=2, space=bass.MemorySpace.PSUM) as den_psum,
            tc.tile_pool(name="outsb", bufs=2) as out_pool,
        ):
            # ---- constants ----
            ones = const_pool.tile([TS, 1], f32)
            nc.vector.memset(ones[:], 1.0)
            # transposed x, loaded per 128-wide d-chunk (one [128, B] DMA
            # each) to keep the HWDGE issue queue short ahead of the
            # projection-weight loads
            xTf = const_pool.tile([128, KC, B], f32, tag="xTf")
            xTt = const_pool.tile([128, KC, B], f32, tag="xTt")
            for c in range(KC):
                nc.sync.dma_start(
                    xTf[:, c, :],
                    x_f[:, c * 128:(c + 1) * 128].rearrange("b p -> p b"))
                nc.sync.dma_start(
                    xTt[:, c, :],
                    x_t[:, c * 128:(c + 1) * 128].rearrange("b p -> p b"))
            bias_sb = {}
            for nm in ("bq", "bk", "bv"):
                t = const_pool.tile([B, WC], f32, tag=nm)
                nc.sync.dma_start(t[:], bs[nm][:].to_broadcast([B, WC]))
                bias_sb[nm] = t

            # ---- head-sharded projections for ALL batches ----
            # ccin rows are global batches = (dest_core, local_batch) — already
            # destination-major, so a flat AllToAll hands every core exactly
            # its own batches' projections from all 8 head-blocks.
            ccin_d = dram_pool.tile([B, 3 * WC], f32, name="ccin_d")
            gout_d = dram_pool.tile([B, 3 * WC], f32, name="gout_d")
            for wi, (wn, bn, xT) in enumerate(
                    (("wq", "bq", xTf), ("wk", "bk", xTt), ("wv", "bv", xTt))):
                wt = w_pool.tile([128, KC, WC], f32, tag="wt")
                nc.sync.dma_start(
                    wt[:], ws[wn][:].rearrange("(c p) m -> p c m", p=128))
                ps = proj_psum.tile([B, WC], f32, name="ps", tag="ps")
                for c in range(KC):
                    nc.tensor.matmul(ps[:], xT[:, c, :], wt[:, c, :],
                                     start=(c == 0), stop=(c == KC - 1))
                sb = proj_sb_pool.tile([B, WC], f32, name="sb", tag=wn)
                nc.vector.tensor_add(sb[:], ps[:], bias_sb[bn][:])
                nc.sync.dma_start(ccin_d[:, wi * WC:(wi + 1) * WC], sb[:])

            nc.gpsimd.collective_compute(
                "AllToAll", mybir.AluOpType.bypass,
                replica_groups=[list(range(NCORES))],
                ins=[ccin_d[:].opt()], outs=[gout_d[:].opt()])
            # received rows are (src_head_block, local_batch);
            # view as [local_batch, head_block, 3*WC]
            gv = gout_d[:].rearrange("(r b) m -> b r m", b=BPC)

            # ---- attention + cache copy, batches interleaved ----
            q_reps, adders, ps_os, ps_ds = [], [], [], []
            for b in range(BPC):
                q_rep = qrep_pool.tile([128, NH], f32)
                nc.sync.dma_start(
                    q_rep[:].rearrange("p (r c) -> p r c", c=WC),
                    gv[b:b + 1, :, 0:WC].to_broadcast([128, NCORES, WC]))
                mi = madd_pool.tile([128, NT], i32, tag="mi")
                nc.sync.dma_start(mi[:], msk[b].rearrange("(c p) -> p c", p=128))
                mf = madd_pool.tile([128, NT], f32, tag="mf")
                nc.vector.tensor_copy(mf[:], mi[:])
                adder = madd_pool.tile([128, NT], f32, tag="adder")
                # (1 - m) * NEG_ADDER == m*(-NEG_ADDER) + NEG_ADDER
                nc.scalar.activation(adder[:], mf[:],
                                     mybir.ActivationFunctionType.Copy,
                                     bias=float(NEG_ADDER), scale=float(-NEG_ADDER))
                q_reps.append(q_rep)
                adders.append(adder)
                ps_os.append(attn_psum.tile([N, NH], f32,
                                            name="ps_o", tag="ps_o"))
                ps_ds.append(den_psum.tile([N, 1], f32,
                                           name="ps_d", tag="ps_d"))

            def epilogue(b):
                # out[n, :] = ps_o[n, n*H:(n+1)*H] / ps_d[n]
                ps_o, ps_d = ps_os[b], ps_ds[b]
                rec = out_pool.tile([N, 1], f32, name="rec", tag="rec")
                nc.vector.reciprocal(rec[:], ps_d[:])
                # copy out of PSUM fused with the 1/denom normalization
                # (ACT Copy with per-partition scale), then gather the
                # diagonal blocks into the output with a DRAM->DRAM
                # strided DMA
                o_sb = out_pool.tile([N, NH], f32, name="o_sb", tag="o_sb")
                nc.scalar.mul(o_sb[:], ps_o[:], rec[:])
                wview = o_dram[b].rearrange("(a c) -> a c", c=_WSTR)
                nc.sync.dma_start(wview[:N, :NH], o_sb[:])
                rview = o_dram[b].rearrange("(a c) -> a c", c=_RSTR)
                nc.sync.dma_start(out[b], rview[:N, :H])

            # step tile first: its copy-out depends on the AllToAll (as does
            # every tile's score path), and putting it early keeps that
            # latency off the kernel tail where it would serialize.
            # Batch 1 runs one slot behind batch 0 so batch 0's epilogue
            # overlaps batch 1's last tile.
            ti_order = [ti_step] + [t for t in range(NT2) if t != ti_step]
            sched = []
            for i in range(NT2 + 1):
                if i < NT2:
                    sched.append((i, 0))
                if i >= 1:
                    sched.append((i - 1, 1))
            for tn, b in sched:
                    ti = ti_order[tn]
                    q_rep, adder = q_reps[b], adders[b]
                    ps_o, ps_d = ps_os[b], ps_ds[b]
                    rows = slice(ti * TST, (ti + 1) * TST)
                    # tile layout [p, cc, nh]: seq t = ti*TST + cc*TS + p
                    k_t = k_pool.tile([128, CPT, NH], f32, tag="k_t")
                    nc.sync.dma_start(
                        k_t[:], ck[b, rows, :].rearrange("(c p) m -> p c m", p=TS))
                    v_t = v_pool.tile([128, CPT, NH], f32, tag="v_t")
                    nc.sync.dma_start(
                        v_t[:], cv[b, rows, :].rearrange("(c p) m -> p c m", p=TS))
                    if ti == ti_step:
                        # step-row update via SWDGE accumulate-DMA (no
                        # partition-alignment constraint, unlike DVE ops)
                        nc.gpsimd.dma_start(
                            k_t[p_step:p_step + 1, cc_step, :],
                            gv[b:b + 1, :, WC:2 * WC],
                            accum_op=mybir.AluOpType.add)
                        nc.gpsimd.dma_start(
                            v_t[p_step:p_step + 1, cc_step, :],
                            gv[b:b + 1, :, 2 * WC:3 * WC],
                            accum_op=mybir.AluOpType.add)
                    # cache copy-out (after the step-row update); issued from
                    # the scalar-engine HWDGE ring so a store waiting on its
                    # load can't head-of-line-block the SP ring's loads
                    nc.scalar.dma_start(
                        kt[b, rows, :].rearrange("(c p) m -> p c m", p=TS), k_t[:])
                    nc.scalar.dma_start(
                        vt[b, rows, :].rearrange("(c p) m -> p c m", p=TS), v_t[:])

                    # scores[t, n] = sum_h k[t, n*H+h] * q[n*H+h]
                    sm = smul_pool.tile([128, CPT, NH], f32, tag="sm")
                    nc.vector.tensor_mul(
                        sm[:], k_t[:],
                        q_rep[:, None, :].broadcast_to([128, CPT, NH]))
                    sc = sc_pool.tile([128, CPT, N], f32, tag="sc")
                    nc.vector.reduce_sum(
                        sc[:], sm[:].rearrange("p c (n h) -> p c n h", h=H),
                        axis=mybir.AxisListType.X)
                    pr = pr_pool.tile([128, CPT, N], f32, tag="pr")
                    for cc in range(CPT):
                        nc.scalar.activation(
                            pr[:, cc, :], sc[:, cc, :],
                            mybir.ActivationFunctionType.Exp,
                            bias=adder[:, CPT * ti + cc:CPT * ti + cc + 1],
                            scale=SCALE)
                        # output / denominator accumulation (probs stationary)
                        first = (tn == 0 and cc == 0)
                        last = (tn == NT2 - 1 and cc == CPT - 1)
                        for h2 in range(2):
                            nc.tensor.matmul(
                                ps_o[:, h2 * 512:(h2 + 1) * 512],
                                pr[:, cc, :], v_t[:, cc, h2 * 512:(h2 + 1) * 512],
                                start=first, stop=last)
                        nc.tensor.matmul(ps_d[:], pr[:, cc, :], ones[:],
                                         start=first, stop=last)
                    if tn == NT2 - 1:
                        epilogue(b)

    nc.compile()
    return nc


def kernel(**inputs):
    global LAST_RESULT
    step = int(np.asarray(inputs["decode_loop_step"]))

    def f32(name, shape):
        return np.ascontiguousarray(
            np.asarray(inputs[name], dtype=np.float32).reshape(shape))

    from_t = f32("from_tensor", (B, D))
    to_t = f32("to_tensor", (B, D))
    mask = np.ascontiguousarray(
        np.asarray(inputs["attention_mask"], dtype=np.int32).reshape(B, S))
    cache_k = f32("cache_key", (B, S, NH))
    cache_v = f32("cache_value", (B, S, NH))
    wq, wk, wv = (f32(nm, (D, NH)) for nm in ("wq", "wk", "wv"))
    bq, bk, bv = (f32(nm, (1, NH)) for nm in ("bq", "bk", "bv"))

    if step not in _BUILD_CACHE:
        _BUILD_CACHE[step] = _build(step)
    nc = _BUILD_CACHE[step]

    in_maps = []
    for c in range(NCORES):
        sl = slice(c * BPC, (c + 1) * BPC)
        wsl = slice(c * WC, (c + 1) * WC)
        in_maps.append({
            "x_f": from_t, "x_t": to_t, "msk": mask[sl],
            "ck": cache_k[sl], "cv": cache_v[sl],
            "wq": np.ascontiguousarray(wq[:, wsl]),
            "wk": np.ascontiguousarray(wk[:, wsl]),
            "wv": np.ascontiguousarray(wv[:, wsl]),
            "bq": np.ascontiguousarray(bq[:, wsl]),
            "bk": np.ascontiguousarray(bk[:, wsl]),
            "bv": np.ascontiguousarray(bv[:, wsl]),
        })

    res = run_bass_kernel_spmd(nc, in_maps, core_ids=list(range(NCORES)))
    LAST_RESULT = res

    out = np.concatenate([r["out"] for r in res.results], axis=0)
    key_t = np.concatenate([r["kt"] for r in res.results], axis=0)
    value_t = np.concatenate([r["vt"] for r in res.results], axis=0)
    return (out.reshape(B, F, N, H),
            key_t.reshape(B, S, N, H),
            value_t.reshape(B, S, N, H))
